# revision 1
# baseline (speedup 1.0000x reference)
import numpy as np
import ml_dtypes

bf16 = ml_dtypes.bfloat16

H = 12
HS = 64
ALL = H * HS          # 768
P = 128
B = 2
S = 1024
C = 64                # output channels (W_out cols)
SCALING = HS ** 0.25  # 2.8284...
S_CORE = 256          # s-rows per core
NSLAB = S_CORE // 8   # 32 slabs of 8 s-rows
NCORES = 8

# D (gather data) layout: [pad0 x3 | qeb[0..254] | pad254 x3] = 261, padded to 264
D_W = 264

_COMPILED = None


def _build_nc(nslab=NSLAB):
    import concourse.bacc as bacc
    import concourse.mybir as mybir
    from concourse.tile import TileContext

    dt = mybir.dt
    AF = mybir.ActivationFunctionType
    ALU = mybir.AluOpType

    nc = bacc.Bacc()

    p1T = nc.dram_tensor("p1T", [ALL, S], dt.bfloat16, kind="ExternalInput")
    p1Tq = nc.dram_tensor("p1Tq", [ALL, S_CORE], dt.bfloat16, kind="ExternalInput")
    wqk = nc.dram_tensor("wqk", [ALL, 2 * ALL], dt.bfloat16, kind="ExternalInput")
    b2d = nc.dram_tensor("b2d", [128, 12], dt.float32, kind="ExternalInput")
    relT = nc.dram_tensor("relT", [128, 256], dt.bfloat16, kind="ExternalInput")
    wfull = nc.dram_tensor("wfull", [128, 512], dt.bfloat16, kind="ExternalInput")
    idxs = nc.dram_tensor("idxs", [128, 16 * NSLAB], dt.uint16, kind="ExternalInput")

    # blocked output: [slab, t_within_chunk, t_chunk, s_within_slab, c]
    out = nc.dram_tensor("out", [NSLAB, 128, 8, 8, C], dt.bfloat16, kind="ExternalOutput")

    inv_s = float(1.0 / SCALING)

    with TileContext(nc) as tc:
        with (
            tc.tile_pool(name="const", bufs=1) as cpool,
            tc.tile_pool(name="persist", bufs=1) as ppool,
            tc.tile_pool(name="slab", bufs=2) as spool,
            tc.tile_pool(name="outp", bufs=3) as opool,
        ):
            # ---- constant loads ----
            wqk_t = cpool.tile([128, 6, 2 * ALL], dt.bfloat16)
            p1T_t = cpool.tile([128, 6, S], dt.bfloat16)
            p1Tq_t = cpool.tile([128, 6, S_CORE], dt.bfloat16)
            wqk_v = wqk.rearrange("(a p) f -> p a f", p=128)
            p1T_v = p1T.rearrange("(a p) s -> p a s", p=128)
            p1Tq_v = p1Tq.rearrange("(a p) s -> p a s", p=128)
            for ca in range(6):
                nc.sync.dma_start(out=wqk_t[:, ca, :], in_=wqk_v[:, ca, :])
                nc.sync.dma_start(out=p1T_t[:, ca, :], in_=p1T_v[:, ca, :])
                nc.sync.dma_start(out=p1Tq_t[:, ca, :], in_=p1Tq_v[:, ca, :])
            b2d_t = cpool.tile([128, 12], dt.float32)
            nc.sync.dma_start(out=b2d_t[:], in_=b2d[:])
            relT_t = cpool.tile([128, 256], dt.bfloat16)
            nc.sync.dma_start(out=relT_t[:], in_=relT[:])
            wfull_t = cpool.tile([128, 512], dt.bfloat16)
            nc.sync.dma_start(out=wfull_t[:], in_=wfull[:])
            idxs_t = cpool.tile([128, 16 * NSLAB], dt.uint16)
            nc.sync.dma_start(out=idxs_t[:], in_=idxs[:])

            # ---- projections: uT = W_qk^T-contract(p1T); kT (all S), qT (own S_CORE) ----
            kT_t = ppool.tile([128, 6, S], dt.bfloat16)    # rows f = 768 + cf*128 + p
            qT_t = ppool.tile([128, 6, S_CORE], dt.bfloat16)
            with tc.tile_pool(name="ps_proj", bufs=2, space="PSUM") as pj_pool:
                for cf in range(6):
                    for th in range(2):
                        pj = pj_pool.tile([128, 512], dt.float32)
                        for ca in range(6):
                            nc.tensor.matmul(
                                pj[:],
                                lhsT=wqk_t[:, ca, ALL + cf * 128 : ALL + cf * 128 + 128],
                                rhs=p1T_t[:, ca, th * 512 : th * 512 + 512],
                                start=(ca == 0),
                                stop=(ca == 5),
                            )
                        nc.scalar.activation(
                            kT_t[:, cf, th * 512 : th * 512 + 512],
                            pj[:],
                            AF.Identity,
                            bias=b2d_t[:, 6 + cf : 7 + cf],
                            scale=inv_s,
                        )
                for cf in range(6):
                    pj = pj_pool.tile([128, 512], dt.float32)
                    for ca in range(6):
                        nc.tensor.matmul(
                            pj[:, 0:S_CORE],
                            lhsT=wqk_t[:, ca, cf * 128 : cf * 128 + 128],
                            rhs=p1Tq_t[:, ca, :],
                            start=(ca == 0),
                            stop=(ca == 5),
                        )
                    nc.scalar.activation(
                        qT_t[:, cf, :],
                        pj[:, 0:S_CORE],
                        AF.Identity,
                        bias=b2d_t[:, cf : cf + 1],
                        scale=inv_s,
                    )

            # persistent per-slab weights tiles (double-buffered manually)
            qsl_t = [ppool.tile([128, 128], dt.bfloat16, name=f"qsl{i}", tag=f"qsl{i}") for i in range(2)]
            qbd_t = [ppool.tile([128, 6, 128], dt.bfloat16, name=f"qbd{i}", tag=f"qbd{i}") for i in range(2)]
            for i in range(2):
                nc.vector.memset(qsl_t[i][:], 0.0)
                nc.vector.memset(qbd_t[i][:], 0.0)

            slab_pools = (
                tc.tile_pool(name="ps_qe", bufs=1, space="PSUM"),
                tc.tile_pool(name="ps_a", bufs=2, space="PSUM"),
                tc.tile_pool(name="ps_o", bufs=3, space="PSUM"),
            )
            pqe_pool = slab_pools[0].__enter__()
            pa_pool = slab_pools[1].__enter__()
            po_pool = slab_pools[2].__enter__()

            # ---- per-slab pipeline (2-stage software pipeline) ----
            bsl_store = {}

            def front(g):
                """q-dependent front-end for slab g: qsl, qe, D, gather, qbd."""
                s0 = 8 * g
                qsl = qsl_t[g % 2]
                qbd = qbd_t[g % 2]

                # qe lhsT: qsl[d-part, grp*16 + 2k + p2] = qT[(2k+p2)*64+d, s0+grp]
                qsl_v = qsl.rearrange("p (g k c) -> p k g c", g=8, c=2)
                for p2 in range(2):
                    r0 = p2 * 64
                    nc.vector.tensor_copy(
                        qsl_v[r0 : r0 + 64, 0:6, :, p2],
                        qT_t[r0 : r0 + 64, :, s0 : s0 + 8],
                    )

                # qe matmul -> [128=(grp,h), 256]
                pqe = pqe_pool.tile([128, 256], dt.float32)
                nc.tensor.matmul(pqe[:], lhsT=qsl[:], rhs=relT_t[:], start=True, stop=True)

                # block-sparse q weights: qbd[c][(h',d) rows, grp*16+h] = qT
                qbd_v = qbd.rearrange("p k (a b) -> p k a b", b=16)
                for c6 in range(6):
                    nc.gpsimd.tensor_copy(
                        qbd_v[0:64, c6, :, 2 * c6],
                        qT_t[0:64, c6, s0 : s0 + 8],
                    )
                    nc.gpsimd.tensor_copy(
                        qbd_v[64:128, c6, :, 2 * c6 + 1],
                        qT_t[64:128, c6, s0 : s0 + 8],
                    )

                # gather data row: [pad0 x3 | qe[129..255] | qe[0..127] | pad254 x3]
                d_t = spool.tile([128, D_W], dt.bfloat16, tag="dgat")
                nc.scalar.activation(d_t[:, 3:130], pqe[:, 129:256], AF.Copy)
                nc.scalar.activation(d_t[:, 130:258], pqe[:, 0:128], AF.Copy)
                for i in range(3):
                    nc.gpsimd.tensor_copy(d_t[:, i : i + 1], d_t[:, 3:4])
                    nc.gpsimd.tensor_copy(d_t[:, 258 + i : 259 + i], d_t[:, 257:258])

                # bias expansion gather: Bsl[p, t] = D[p, idx(t)+0..3]
                bsl = spool.tile([128, S], dt.bfloat16, tag="bsl")
                nc.gpsimd.indirect_copy(
                    bsl.rearrange("p (n i) -> p n i", i=4),
                    d_t.rearrange("p (n i) -> p n i", i=4),
                    idxs_t[:, g * 16 : g * 16 + 16],
                    True,
                )
                bsl_store[g] = bsl

            def back(g):
                """scores + final + store for slab g."""
                s0 = 8 * g
                qbd = qbd_t[g % 2]
                bsl = bsl_store.pop(g)

                # scores + bias add -> A [128=(grp,h), 1024] bf16
                a_t = spool.tile([128, S], dt.bfloat16, tag="a")
                pa0 = pa_pool.tile([128, 512], dt.float32, name="pa0", tag="pa0")
                pa1 = pa_pool.tile([128, 512], dt.float32, name="pa1", tag="pa1")
                pas = (pa0, pa1)
                for c6 in range(6):
                    for th in range(2):
                        nc.tensor.matmul(
                            pas[th][:],
                            lhsT=qbd[:, c6, :],
                            rhs=kT_t[:, c6, th * 512 : th * 512 + 512],
                            start=(c6 == 0),
                            stop=(c6 == 5),
                        )
                for th in range(2):
                    nc.vector.tensor_tensor(
                        a_t[:, th * 512 : th * 512 + 512],
                        pas[th][:],
                        bsl[:, th * 512 : th * 512 + 512],
                        op=ALU.add,
                    )

                # final W_out contraction with A as stationary weights:
                # out[t, (s_local, c)] = sum_{(grp,h)} A[(grp,h), t] * Wfull[(grp,h), (s_local,c)]
                # staging layout [t-part, (u, s, c)]: contiguous copy dests
                so = opool.tile([128, 8, 8, 64], dt.bfloat16, tag="so")
                for tc_i in range(8):
                    po = po_pool.tile([128, 512], dt.float32)
                    nc.tensor.matmul(
                        po[:],
                        lhsT=a_t[:, tc_i * 128 : tc_i * 128 + 128],
                        rhs=wfull_t[:],
                        start=True,
                        stop=True,
                    )
                    so_dst = so[:, tc_i, :, :].rearrange("p a b -> p (a b)")
                    if tc_i % 2 == 0:
                        nc.scalar.activation(so_dst, po[:], AF.Copy)
                    else:
                        nc.vector.tensor_copy(so_dst, po[:])
                nc.sync.dma_start(out=out[g], in_=so[:])

            front(0)
            for g in range(nslab):
                if g + 1 < nslab:
                    front(g + 1)
                back(g)

            for cm in reversed(slab_pools):
                cm.__exit__(None, None, None)

    nc.finalize()
    return nc


def _host_prep(p1, W_qk, b_qk, rel_emb, W_out, b_out):
    wqk_bf = W_qk.astype(bf16)
    b2d = np.ascontiguousarray((b_qk / SCALING).reshape(12, 128).T.astype(np.float32))
    relT1 = np.ascontiguousarray((rel_emb[:256] / SCALING).T).astype(bf16)
    relT = np.concatenate([relT1, relT1], axis=0)
    wfull = np.zeros((128, 512), np.float32)
    for grp in range(8):
        wfull[grp * 16 : grp * 16 + H, grp * 64 : grp * 64 + 64] = W_out
    wfull = wfull.astype(bf16)

    p1T = [np.ascontiguousarray(p1[b].T).astype(bf16) for b in range(B)]

    in_maps = []
    for core in range(NCORES):
        b = core // 4
        s_off = (core % 4) * S_CORE
        p1Tq = np.ascontiguousarray(p1T[b][:, s_off : s_off + S_CORE])

        # gather index table: [128, 16*NSLAB] uint16
        idxs = np.zeros((128, 16 * NSLAB), np.uint16)
        i_arr = np.arange(256)
        for g in range(NSLAB):
            for grp in range(8):
                s_glob = s_off + 8 * g + grp
                j = 4 * i_arr - s_glob + 127
                idx = np.clip(j, -3, 254) + 3
                idxs[16 * grp + (i_arr % 16), g * 16 + i_arr // 16] = idx.astype(
                    np.uint16
                )

        in_maps.append(
            {
                "p1T": p1T[b],
                "p1Tq": p1Tq,
                "wqk": wqk_bf,
                "b2d": b2d,
                "relT": relT,
                "wfull": wfull,
                "idxs": idxs,
            }
        )
    return in_maps


def _make_runner():
    """Build the bass module once and return a callable(in_maps) -> list of
    per-core output arrays, with a persistently cached jitted executable."""
    import jax
    import jax.numpy as jnp
    from jax.experimental.shard_map import shard_map
    from jax.sharding import Mesh, NamedSharding, PartitionSpec

    from concourse import bass2jax
    from concourse.bass2jax import _bass_exec_p, install_neuronx_cc_hook

    install_neuronx_cc_hook()
    nc = _build_nc()

    in_names = ["p1T", "p1Tq", "wqk", "b2d", "relT", "wfull", "idxs"]
    out_name = "out"
    out_shape = (NSLAB, 128, 8, 8, C)
    partition_name = nc.partition_id_tensor.name if nc.partition_id_tensor else None
    out_aval = jax.core.ShapedArray(out_shape, np.dtype(bf16))

    all_in_names = list(in_names) + [out_name]
    if partition_name is not None:
        all_in_names.append(partition_name)

    def _body(*args):
        operands = list(args)
        if partition_name is not None:
            operands.append(bass2jax.partition_id_tensor())
        outs = _bass_exec_p.bind(
            *operands,
            out_avals=(out_aval,),
            in_names=tuple(all_in_names),
            out_names=(out_name,),
            lowering_input_output_aliases=(),
            sim_require_finite=True,
            sim_require_nnan=True,
            nc=nc,
        )
        return tuple(outs)

    devices = jax.devices()[:NCORES]
    mesh = Mesh(np.asarray(devices), ("core",))
    n_params = len(in_names)
    in_specs = (PartitionSpec("core"),) * (n_params + 1)
    out_specs = (PartitionSpec("core"),)
    sharded = jax.jit(
        shard_map(_body, mesh=mesh, in_specs=in_specs, out_specs=out_specs,
                  check_rep=False),
        donate_argnums=(n_params,),
        keep_unused=True,
    )
    sharding = NamedSharding(mesh, PartitionSpec("core"))
    zeros_fn = jax.jit(
        lambda: jnp.zeros((NCORES * NSLAB, 128, 8, 8, C), np.dtype(bf16)),
        out_shardings=sharding,
    )

    def run(in_maps, out_f32):
        """Execute and write fp32 results directly into out_f32 [B,S,S,C]."""
        concat_in = [
            np.concatenate([np.asarray(m[name]) for m in in_maps], axis=0)
            for name in in_names
        ]
        zero_out = zeros_fn()
        out_arr = sharded(*concat_in, zero_out)[0]

        shards = sorted(out_arr.addressable_shards, key=lambda sh: sh.index[0].start)

        def convert(core, raw):
            b = core // 4
            s_off = (core % 4) * S_CORE
            # fast bf16 -> f32 upcast, then unblock (g,t,u,s,c) -> (s_glob, t_glob, c)
            u = raw.view(np.uint16).astype(np.uint32) << 16
            f = u.view(np.float32)
            out_f32[b, s_off : s_off + S_CORE] = (
                f.transpose(0, 3, 2, 1, 4).reshape(S_CORE, S, C)
            )

        # downloads are serial (tunnel-bound); hide the upcast/detranspose
        # behind the next shard's download via a single worker thread
        from concurrent.futures import ThreadPoolExecutor

        with ThreadPoolExecutor(1) as ex:
            futs = []
            for core, sh in enumerate(shards):
                raw = np.asarray(sh.data)  # network-bound fetch
                futs.append(ex.submit(convert, core, raw))
            for fu in futs:
                fu.result()

    return run


def kernel(p0, p1, p2, W_qk, b_qk, rel_emb, W_out, b_out):
    global _COMPILED

    p1 = np.asarray(p1, np.float32)
    W_qk = np.asarray(W_qk, np.float32)
    b_qk = np.asarray(b_qk, np.float32)
    rel_emb = np.asarray(rel_emb, np.float32)
    W_out = np.asarray(W_out, np.float32)
    b_out = np.asarray(b_out, np.float32)

    if _COMPILED is None:
        _COMPILED = _make_runner()
    run = _COMPILED

    in_maps = _host_prep(p1, W_qk, b_qk, rel_emb, W_out, b_out)
    full = np.empty((B, S, S, C), np.float32)
    run(in_maps, full)
    if np.any(b_out):
        full += np.asarray(b_out, np.float32)
    return full



# revision 2
# speedup vs baseline: 8.0791x; 8.0791x over previous
import numpy as np
import ml_dtypes

bf16 = ml_dtypes.bfloat16

H = 12
HS = 64
ALL = H * HS          # 768
P = 128
B = 2
S = 1024
C = 64                # output channels (W_out cols)
SCALING = HS ** 0.25  # 2.8284...
S_CORE = 256          # s-rows per core
NSLAB = S_CORE // 8   # 32 slabs of 8 s-rows
NCORES = 8

IN_NAMES = ["p1T", "p1Tq", "wqk", "b2d", "relPad2", "wfull"]

_COMPILED = None


def _build_nc(nslab=NSLAB):
    import concourse.bacc as bacc
    import concourse.mybir as mybir
    from concourse.tile import TileContext

    dt = mybir.dt
    AF = mybir.ActivationFunctionType

    nc = bacc.Bacc()

    p1T = nc.dram_tensor("p1T", [ALL, S], dt.bfloat16, kind="ExternalInput")
    p1Tq = nc.dram_tensor("p1Tq", [ALL, S_CORE], dt.bfloat16, kind="ExternalInput")
    wqk = nc.dram_tensor("wqk", [ALL, 2 * ALL], dt.bfloat16, kind="ExternalInput")
    b2d = nc.dram_tensor("b2d", [128, 12], dt.float32, kind="ExternalInput")
    relPad2 = nc.dram_tensor("relPad2", [128, 2048], dt.bfloat16, kind="ExternalInput")
    wfull = nc.dram_tensor("wfull", [128, 512], dt.bfloat16, kind="ExternalInput")

    # blocked output: [slab, t_within_chunk, t_chunk, s_within_slab, c]
    out = nc.dram_tensor("out", [NSLAB, 128, 8, 8, C], dt.bfloat16, kind="ExternalOutput")

    inv_s = float(1.0 / SCALING)

    with TileContext(nc) as tc:
        with (
            tc.tile_pool(name="const", bufs=1) as cpool,
            tc.tile_pool(name="persist", bufs=1) as ppool,
            tc.tile_pool(name="slab", bufs=2) as spool,
            tc.tile_pool(name="outp", bufs=3) as opool,
        ):
            # ---- constant loads ----
            wqk_t = cpool.tile([128, 6, 2 * ALL], dt.bfloat16)
            p1T_t = cpool.tile([128, 6, S], dt.bfloat16)
            p1Tq_t = cpool.tile([128, 6, S_CORE], dt.bfloat16)
            wqk_v = wqk.rearrange("(a p) f -> p a f", p=128)
            p1T_v = p1T.rearrange("(a p) s -> p a s", p=128)
            p1Tq_v = p1Tq.rearrange("(a p) s -> p a s", p=128)
            for ca in range(6):
                nc.sync.dma_start(out=wqk_t[:, ca, :], in_=wqk_v[:, ca, :])
                nc.sync.dma_start(out=p1T_t[:, ca, :], in_=p1T_v[:, ca, :])
                nc.sync.dma_start(out=p1Tq_t[:, ca, :], in_=p1Tq_v[:, ca, :])
            b2d_t = cpool.tile([128, 12], dt.float32)
            nc.sync.dma_start(out=b2d_t[:], in_=b2d[:])
            rel_t = cpool.tile([128, 2048], dt.bfloat16)
            nc.sync.dma_start(out=rel_t[:], in_=relPad2[:])
            wfull_t = cpool.tile([128, 512], dt.bfloat16)
            nc.sync.dma_start(out=wfull_t[:], in_=wfull[:])

            # ---- projections: kT (all S), qT (own S_CORE) ----
            kT_t = ppool.tile([128, 6, S], dt.bfloat16)    # rows f = 768 + cf*128 + p
            qT_t = ppool.tile([128, 6, S_CORE], dt.bfloat16)
            with tc.tile_pool(name="ps_proj", bufs=2, space="PSUM") as pj_pool:
                for cf in range(6):
                    for th in range(2):
                        pj = pj_pool.tile([128, 512], dt.float32)
                        for ca in range(6):
                            nc.tensor.matmul(
                                pj[:],
                                lhsT=wqk_t[:, ca, ALL + cf * 128 : ALL + cf * 128 + 128],
                                rhs=p1T_t[:, ca, th * 512 : th * 512 + 512],
                                start=(ca == 0),
                                stop=(ca == 5),
                            )
                        nc.scalar.activation(
                            kT_t[:, cf, th * 512 : th * 512 + 512],
                            pj[:],
                            AF.Identity,
                            bias=b2d_t[:, 6 + cf : 7 + cf],
                            scale=inv_s,
                        )
                for cf in range(6):
                    pj = pj_pool.tile([128, 512], dt.float32)
                    for ca in range(6):
                        nc.tensor.matmul(
                            pj[:, 0:S_CORE],
                            lhsT=wqk_t[:, ca, cf * 128 : cf * 128 + 128],
                            rhs=p1Tq_t[:, ca, :],
                            start=(ca == 0),
                            stop=(ca == 5),
                        )
                    nc.scalar.activation(
                        qT_t[:, cf, :],
                        pj[:, 0:S_CORE],
                        AF.Identity,
                        bias=b2d_t[:, cf : cf + 1],
                        scale=inv_s,
                    )

            # ---- bulk lhsT staging for all slabs ----
            # scores lhsT: qbd_all[p=(p2,d), c6, col = 128*g + 16*a + (2*c6+p2)]
            #   = q[h=2*c6+p2, d, s = 8*g + a]
            qbd_all = ppool.tile([128, 6, 128 * NSLAB], dt.bfloat16)
            qv = qbd_all.rearrange("p k (g a b) -> p k g a b", a=8, b=16)
            nc.vector.memset(qbd_all.rearrange("p k x -> p (k x)"), 0.0)
            for c6 in range(6):
                for p2 in range(2):
                    r0 = 64 * p2
                    nc.vector.tensor_copy(
                        qv[r0 : r0 + 64, c6, :, :, 2 * c6 + p2],
                        qT_t[r0 : r0 + 64, c6, :].rearrange("p (g a) -> p g a", a=8),
                    )

            # bias lhsT: qbp_all[p=(v,d), col = 512*g + 128*j + 16*(2*j+v) + h]
            #   = q[h, d, s = 8*g + 2*j + v]
            # staged via qTds[p=(p2,d), col = 6*s + cf] = q[h=2*cf+p2, d, s]
            qTds = ppool.tile([128, 6 * S_CORE], dt.bfloat16)
            qTds_v = qTds.rearrange("p (s k) -> p s k", k=6)
            for cf in range(6):
                nc.scalar.activation(qTds_v[:, :, cf], qT_t[:, cf, :], AF.Copy)

            qbp_all = ppool.tile([128, 512 * NSLAB], dt.bfloat16)
            nc.gpsimd.memset(qbp_all[:], 0.0)
            qbp_v = qbp_all.rearrange("p (g r two) -> p g r two", r=256, two=2)
            qTds_s = qTds.rearrange("p (g r k) -> p g r k", r=8, k=6)
            for v in range(2):
                for j in range(4):
                    c0h = (160 * j + 16 * v) // 2
                    for p2 in range(2):
                        nc.vector.tensor_copy(
                            qbp_v[64 * v : 64 * v + 64, :, c0h : c0h + 6, p2],
                            qTds_s[64 * p2 : 64 * p2 + 64, :, 2 * j + v, :],
                        )

            slab_pools = (
                tc.tile_pool(name="ps_a", bufs=2, space="PSUM"),
                tc.tile_pool(name="ps_o", bufs=3, space="PSUM"),
            )
            pa_pool = slab_pools[0].__enter__()
            po_pool = slab_pools[1].__enter__()

            # ---- per-slab pipeline ----
            def slab(g):
                s0 = 8 * g
                a_t = spool.tile([128, S], dt.bfloat16, tag="a")
                pa0 = pa_pool.tile([128, 512], dt.float32, name="pa0", tag="pa0")
                pa1 = pa_pool.tile([128, 512], dt.float32, name="pa1", tag="pa1")
                pas = (pa0, pa1)
                # scores: accumulate over 6 head-pair chunks of W_qk features
                for c6 in range(6):
                    for th in range(2):
                        nc.tensor.matmul(
                            pas[th][:],
                            lhsT=qbd_all[:, c6, 128 * g : 128 * g + 128],
                            rhs=kT_t[:, c6, th * 512 : th * 512 + 512],
                            start=(c6 == 0),
                            stop=False,
                        )
                # rel-pos bias: 4 band-pair matmuls against shifted views of
                # the edge-clamped rel table (shift = compile-time col offset)
                for j in range(4):
                    c0 = 1024 - (s0 + 2 * j)
                    for th in range(2):
                        nc.tensor.matmul(
                            pas[th][:],
                            lhsT=qbp_all[:, 512 * g + 128 * j : 512 * g + 128 * j + 128],
                            rhs=rel_t[:, c0 + th * 512 : c0 + th * 512 + 512],
                            start=False,
                            stop=(j == 3),
                        )
                nc.scalar.activation(a_t[:, 0:512], pa0[:], AF.Copy)
                nc.vector.tensor_copy(a_t[:, 512:1024], pa1[:])

                # final W_out contraction with A as stationary weights
                so = opool.tile([128, 8, 8, 64], dt.bfloat16, tag="so")
                for tc_i in range(8):
                    po = po_pool.tile([128, 512], dt.float32)
                    nc.tensor.matmul(
                        po[:],
                        lhsT=a_t[:, tc_i * 128 : tc_i * 128 + 128],
                        rhs=wfull_t[:],
                        start=True,
                        stop=True,
                    )
                    so_dst = so[:, tc_i, :, :].rearrange("p a b -> p (a b)")
                    if tc_i % 2 == 0:
                        nc.scalar.activation(so_dst, po[:], AF.Copy)
                    else:
                        nc.vector.tensor_copy(so_dst, po[:])
                nc.sync.dma_start(out=out[g], in_=so[:])

            for g in range(nslab):
                slab(g)

            for cm in reversed(slab_pools):
                cm.__exit__(None, None, None)

    nc.finalize()
    return nc


def _host_prep(p1, W_qk, b_qk, rel_emb, W_out, b_out):
    wqk_bf = W_qk.astype(bf16)
    b2d = np.ascontiguousarray((b_qk / SCALING).reshape(12, 128).T.astype(np.float32))
    rel_scaled = (rel_emb / SCALING).astype(np.float32)  # [258, 64]
    wfull = np.zeros((128, 512), np.float32)
    for grp in range(8):
        wfull[grp * 16 : grp * 16 + H, grp * 64 : grp * 64 + 64] = W_out
    wfull = wfull.astype(bf16)

    p1T = [np.ascontiguousarray(p1[b].T).astype(bf16) for b in range(B)]

    in_maps = []
    u = np.arange(2048)
    for core in range(NCORES):
        b = core // 4
        s_off = (core % 4) * S_CORE
        p1Tq = np.ascontiguousarray(p1T[b][:, s_off : s_off + S_CORE])

        # relPad2[64*v + d, u] = rel_scaled[row(clip(u - s_off - 1024 - v))][d]
        # (per-core origin baked in so one SPMD NEFF serves all cores)
        relPad2 = np.empty((128, 2048), np.float32)
        for v in range(2):
            e = np.clip(u - s_off - 1024 - v, -127, 127)
            rows = np.where(e >= 0, e, e + 256)
            relPad2[64 * v : 64 * v + 64, :] = rel_scaled[rows, :].T
        relPad2 = relPad2.astype(bf16)

        in_maps.append(
            {
                "p1T": p1T[b],
                "p1Tq": p1Tq,
                "wqk": wqk_bf,
                "b2d": b2d,
                "relPad2": relPad2,
                "wfull": wfull,
            }
        )
    return in_maps


class _Runner:
    """Holds the AOT fast-dispatch executable (C++ pjit fast path, no
    donated output: the kernel writes every out element, so PJRT's
    uninitialized result allocation is safe)."""

    def __init__(self):
        import jax
        from jax.experimental.shard_map import shard_map
        from jax.sharding import Mesh, NamedSharding, PartitionSpec

        from concourse import bass2jax
        from concourse.bass2jax import (
            _bass_exec_p,
            fast_dispatch_compile,
            install_neuronx_cc_hook,
        )

        install_neuronx_cc_hook()
        nc = _build_nc()
        self.nc = nc

        out_aval = jax.core.ShapedArray((NSLAB, 128, 8, 8, C), np.dtype(bf16))
        pname = nc.partition_id_tensor.name if nc.partition_id_tensor else None
        all_in = list(IN_NAMES) + ([pname] if pname else [])

        def _body(*args):
            operands = list(args)
            if pname:
                operands.append(bass2jax.partition_id_tensor())
            outs = _bass_exec_p.bind(
                *operands,
                out_avals=(out_aval,),
                in_names=tuple(all_in),
                out_names=("out",),
                lowering_input_output_aliases=(),
                sim_require_finite=True,
                sim_require_nnan=True,
                nc=nc,
            )
            return tuple(outs)

        devices = jax.devices()[:NCORES]
        self.mesh = Mesh(np.asarray(devices), ("core",))
        self.sharding = NamedSharding(self.mesh, PartitionSpec("core"))
        in_shapes = {
            "p1T": (ALL, S), "p1Tq": (ALL, S_CORE), "wqk": (ALL, 2 * ALL),
            "b2d": (128, 12), "relPad2": (128, 2048), "wfull": (128, 512),
        }
        in_dtypes = {n: np.dtype(bf16) for n in IN_NAMES}
        in_dtypes["b2d"] = np.dtype(np.float32)
        arg_structs = [
            jax.ShapeDtypeStruct(
                (NCORES * in_shapes[n][0],) + in_shapes[n][1:],
                in_dtypes[n], sharding=self.sharding)
            for n in IN_NAMES
        ]
        self.fast = fast_dispatch_compile(
            lambda: jax.jit(
                shard_map(
                    _body, mesh=self.mesh,
                    in_specs=(PartitionSpec("core"),) * len(IN_NAMES),
                    out_specs=(PartitionSpec("core"),), check_rep=False),
                keep_unused=True,
            ).lower(*arg_structs).compile()
        )
        self._jax = jax

    def device_inputs(self, in_maps):
        jax = self._jax
        concat_in = [
            np.concatenate([np.asarray(m[name]) for m in in_maps], axis=0)
            for name in IN_NAMES
        ]
        return [jax.device_put(a, self.sharding) for a in concat_in]

    def run(self, in_maps, out_f32):
        """Execute and write fp32 results directly into out_f32 [B,S,S,C]."""
        dev_in = self.device_inputs(in_maps)
        out_arr = self.fast(*dev_in)[0]

        shards = sorted(out_arr.addressable_shards, key=lambda sh: sh.index[0].start)

        def convert(core, raw):
            b = core // 4
            s_off = (core % 4) * S_CORE
            # fast bf16 -> f32 upcast, then unblock (g,t,u,s,c) -> (s_glob, t_glob, c)
            u = raw.view(np.uint16).astype(np.uint32) << 16
            f = u.view(np.float32)
            out_f32[b, s_off : s_off + S_CORE] = (
                f.transpose(0, 3, 2, 1, 4).reshape(S_CORE, S, C)
            )

        # downloads are serial (tunnel-bound); hide the upcast/detranspose
        # behind the next shard's download via a single worker thread
        from concurrent.futures import ThreadPoolExecutor

        with ThreadPoolExecutor(1) as ex:
            futs = []
            for core, sh in enumerate(shards):
                raw = np.asarray(sh.data)  # network-bound fetch
                futs.append(ex.submit(convert, core, raw))
            for fu in futs:
                fu.result()


def kernel(p0, p1, p2, W_qk, b_qk, rel_emb, W_out, b_out):
    global _COMPILED

    p1 = np.asarray(p1, np.float32)
    W_qk = np.asarray(W_qk, np.float32)
    b_qk = np.asarray(b_qk, np.float32)
    rel_emb = np.asarray(rel_emb, np.float32)
    W_out = np.asarray(W_out, np.float32)
    b_out = np.asarray(b_out, np.float32)

    if _COMPILED is None:
        _COMPILED = _Runner()
    run = _COMPILED

    in_maps = _host_prep(p1, W_qk, b_qk, rel_emb, W_out, b_out)
    full = np.empty((B, S, S, C), np.float32)
    run.run(in_maps, full)
    if np.any(b_out):
        full += np.asarray(b_out, np.float32)
    return full


# revision 4
# speedup vs baseline: 9.6282x; 1.1917x over previous
import numpy as np
import ml_dtypes

bf16 = ml_dtypes.bfloat16

H = 12
HS = 64
ALL = H * HS          # 768
P = 128
B = 2
S = 1024
C = 64                # output channels (W_out cols)
SCALING = HS ** 0.25  # 2.8284...
S_CORE = 256          # s-rows per core
NSLAB = S_CORE // 8   # 32 slabs of 8 s-rows
NCORES = 8

IN_NAMES = ["p1T", "p1Tq", "wqk", "b2d", "relPad2", "wfull"]

_COMPILED = None


def _build_nc(nslab=NSLAB):
    import concourse.bacc as bacc
    import concourse.mybir as mybir
    from concourse.tile import TileContext

    dt = mybir.dt
    AF = mybir.ActivationFunctionType

    nc = bacc.Bacc()

    p1T = nc.dram_tensor("p1T", [ALL, S], dt.bfloat16, kind="ExternalInput")
    p1Tq = nc.dram_tensor("p1Tq", [ALL, S_CORE], dt.bfloat16, kind="ExternalInput")
    wqk = nc.dram_tensor("wqk", [ALL, 2 * ALL], dt.bfloat16, kind="ExternalInput")
    b2d = nc.dram_tensor("b2d", [128, 12], dt.float32, kind="ExternalInput")
    relPad2 = nc.dram_tensor("relPad2", [128, 2048], dt.bfloat16, kind="ExternalInput")
    wfull = nc.dram_tensor("wfull", [128, 512], dt.bfloat16, kind="ExternalInput")

    # blocked output: [slab, t_within_chunk, t_chunk, s_within_slab, c]
    out = nc.dram_tensor("out", [NSLAB, 128, 8, 8, C], dt.bfloat16, kind="ExternalOutput")

    inv_s = float(1.0 / SCALING)

    with TileContext(nc) as tc:
        with (
            tc.tile_pool(name="const", bufs=1) as cpool,
            tc.tile_pool(name="persist", bufs=1) as ppool,
            tc.tile_pool(name="slab", bufs=2) as spool,
            tc.tile_pool(name="outp", bufs=3) as opool,
        ):
            # ---- constant loads ----
            wqk_t = cpool.tile([128, 6, 2 * ALL], dt.bfloat16)
            p1T_t = cpool.tile([128, 6, S], dt.bfloat16)
            p1Tq_t = cpool.tile([128, 6, S_CORE], dt.bfloat16)
            wqk_v = wqk.rearrange("(a p) f -> p a f", p=128)
            p1T_v = p1T.rearrange("(a p) s -> p a s", p=128)
            p1Tq_v = p1Tq.rearrange("(a p) s -> p a s", p=128)
            for ca in range(6):
                nc.sync.dma_start(out=wqk_t[:, ca, :], in_=wqk_v[:, ca, :])
                nc.sync.dma_start(out=p1T_t[:, ca, :], in_=p1T_v[:, ca, :])
                nc.sync.dma_start(out=p1Tq_t[:, ca, :], in_=p1Tq_v[:, ca, :])
            b2d_t = cpool.tile([128, 12], dt.float32)
            nc.sync.dma_start(out=b2d_t[:], in_=b2d[:])
            rel_t = cpool.tile([128, 2048], dt.bfloat16)
            nc.sync.dma_start(out=rel_t[:], in_=relPad2[:])
            wfull_t = cpool.tile([128, 512], dt.bfloat16)
            nc.sync.dma_start(out=wfull_t[:], in_=wfull[:])

            # ---- projections: kT (all S), qT (own S_CORE) ----
            kT_t = ppool.tile([128, 6, S], dt.bfloat16)    # rows f = 768 + cf*128 + p
            qT_t = ppool.tile([128, 6, S_CORE], dt.bfloat16)
            with tc.tile_pool(name="ps_proj", bufs=2, space="PSUM") as pj_pool:
                for cf in range(6):
                    for th in range(2):
                        pj = pj_pool.tile([128, 512], dt.float32)
                        for ca in range(6):
                            nc.tensor.matmul(
                                pj[:],
                                lhsT=wqk_t[:, ca, ALL + cf * 128 : ALL + cf * 128 + 128],
                                rhs=p1T_t[:, ca, th * 512 : th * 512 + 512],
                                start=(ca == 0),
                                stop=(ca == 5),
                            )
                        nc.scalar.activation(
                            kT_t[:, cf, th * 512 : th * 512 + 512],
                            pj[:],
                            AF.Identity,
                            bias=b2d_t[:, 6 + cf : 7 + cf],
                            scale=inv_s,
                        )
                for cf in range(6):
                    pj = pj_pool.tile([128, 512], dt.float32)
                    for ca in range(6):
                        nc.tensor.matmul(
                            pj[:, 0:S_CORE],
                            lhsT=wqk_t[:, ca, cf * 128 : cf * 128 + 128],
                            rhs=p1Tq_t[:, ca, :],
                            start=(ca == 0),
                            stop=(ca == 5),
                        )
                    nc.scalar.activation(
                        qT_t[:, cf, :],
                        pj[:, 0:S_CORE],
                        AF.Identity,
                        bias=b2d_t[:, cf : cf + 1],
                        scale=inv_s,
                    )

            # ---- bulk lhsT staging for all slabs ----
            # scores lhsT: qbd_all[p=(p2,d), c6, col = 128*g + 16*a + (2*c6+p2)]
            #   = q[h=2*c6+p2, d, s = 8*g + a]
            qbd_all = ppool.tile([128, 6, 128 * NSLAB], dt.bfloat16)
            qv = qbd_all.rearrange("p k (g a b) -> p k g a b", a=8, b=16)
            nc.vector.memset(qbd_all.rearrange("p k x -> p (k x)"), 0.0)
            for c6 in range(6):
                for p2 in range(2):
                    r0 = 64 * p2
                    nc.vector.tensor_copy(
                        qv[r0 : r0 + 64, c6, :, :, 2 * c6 + p2],
                        qT_t[r0 : r0 + 64, c6, :].rearrange("p (g a) -> p g a", a=8),
                    )

            # bias lhsT: qbp_all[p=(v,d), col = 512*g + 128*j + 16*(2*j+v) + h]
            #   = q[h, d, s = 8*g + 2*j + v]
            # staged via qTds[p=(p2,d), col = 6*s + cf] = q[h=2*cf+p2, d, s]
            qTds = ppool.tile([128, 6 * S_CORE], dt.bfloat16)
            qTds_v = qTds.rearrange("p (s k) -> p s k", k=6)
            for cf in range(6):
                nc.scalar.activation(qTds_v[:, :, cf], qT_t[:, cf, :], AF.Copy)

            qbp_all = ppool.tile([128, 512 * NSLAB], dt.bfloat16)
            nc.gpsimd.memset(qbp_all[:], 0.0)
            qbp_v = qbp_all.rearrange("p (g r two) -> p g r two", r=256, two=2)
            qTds_s = qTds.rearrange("p (g r k) -> p g r k", r=8, k=6)
            for v in range(2):
                for j in range(4):
                    c0h = (160 * j + 16 * v) // 2
                    for p2 in range(2):
                        nc.vector.tensor_copy(
                            qbp_v[64 * v : 64 * v + 64, :, c0h : c0h + 6, p2],
                            qTds_s[64 * p2 : 64 * p2 + 64, :, 2 * j + v, :],
                        )

            slab_pools = (
                tc.tile_pool(name="ps_a", bufs=2, space="PSUM"),
                tc.tile_pool(name="ps_o", bufs=3, space="PSUM"),
            )
            pa_pool = slab_pools[0].__enter__()
            po_pool = slab_pools[1].__enter__()

            # ---- per-slab pipeline ----
            def slab(g):
                s0 = 8 * g
                a_t = spool.tile([128, S], dt.bfloat16, tag="a")
                pa0 = pa_pool.tile([128, 512], dt.float32, name="pa0", tag="pa0")
                pa1 = pa_pool.tile([128, 512], dt.float32, name="pa1", tag="pa1")
                pas = (pa0, pa1)
                # scores: accumulate over 6 head-pair chunks of W_qk features
                for c6 in range(6):
                    for th in range(2):
                        nc.tensor.matmul(
                            pas[th][:],
                            lhsT=qbd_all[:, c6, 128 * g : 128 * g + 128],
                            rhs=kT_t[:, c6, th * 512 : th * 512 + 512],
                            start=(c6 == 0),
                            stop=False,
                        )
                # rel-pos bias: 4 band-pair matmuls against shifted views of
                # the edge-clamped rel table (shift = compile-time col offset)
                for j in range(4):
                    c0 = 1024 - (s0 + 2 * j)
                    for th in range(2):
                        nc.tensor.matmul(
                            pas[th][:],
                            lhsT=qbp_all[:, 512 * g + 128 * j : 512 * g + 128 * j + 128],
                            rhs=rel_t[:, c0 + th * 512 : c0 + th * 512 + 512],
                            start=False,
                            stop=(j == 3),
                        )
                nc.scalar.activation(a_t[:, 0:512], pa0[:], AF.Copy)
                nc.vector.tensor_copy(a_t[:, 512:1024], pa1[:])

                # final W_out contraction with A as stationary weights
                so = opool.tile([128, 8, 8, 64], dt.bfloat16, tag="so")
                for tc_i in range(8):
                    po = po_pool.tile([128, 512], dt.float32)
                    nc.tensor.matmul(
                        po[:],
                        lhsT=a_t[:, tc_i * 128 : tc_i * 128 + 128],
                        rhs=wfull_t[:],
                        start=True,
                        stop=True,
                    )
                    so_dst = so[:, tc_i, :, :].rearrange("p a b -> p (a b)")
                    if tc_i % 2 == 0:
                        nc.scalar.activation(so_dst, po[:], AF.Copy)
                    else:
                        nc.vector.tensor_copy(so_dst, po[:])
                nc.sync.dma_start(out=out[g], in_=so[:])

            for g in range(nslab):
                slab(g)

            for cm in reversed(slab_pools):
                cm.__exit__(None, None, None)

    nc.finalize()
    return nc


def _host_prep(p1, W_qk, b_qk, rel_emb, W_out, b_out):
    wqk_bf = W_qk.astype(bf16)
    b2d = np.ascontiguousarray((b_qk / SCALING).reshape(12, 128).T.astype(np.float32))
    rel_scaled = (rel_emb / SCALING).astype(np.float32)  # [258, 64]
    wfull = np.zeros((128, 512), np.float32)
    for grp in range(8):
        wfull[grp * 16 : grp * 16 + H, grp * 64 : grp * 64 + 64] = W_out
    wfull = wfull.astype(bf16)

    p1T = [np.ascontiguousarray(p1[b].T).astype(bf16) for b in range(B)]

    in_maps = []
    u = np.arange(2048)
    for core in range(NCORES):
        b = core // 4
        s_off = (core % 4) * S_CORE
        p1Tq = np.ascontiguousarray(p1T[b][:, s_off : s_off + S_CORE])

        # relPad2[64*v + d, u] = rel_scaled[row(clip(u - s_off - 1024 - v))][d]
        # (per-core origin baked in so one SPMD NEFF serves all cores)
        relPad2 = np.empty((128, 2048), np.float32)
        for v in range(2):
            e = np.clip(u - s_off - 1024 - v, -127, 127)
            rows = np.where(e >= 0, e, e + 256)
            relPad2[64 * v : 64 * v + 64, :] = rel_scaled[rows, :].T
        relPad2 = relPad2.astype(bf16)

        in_maps.append(
            {
                "p1T": p1T[b],
                "p1Tq": p1Tq,
                "wqk": wqk_bf,
                "b2d": b2d,
                "relPad2": relPad2,
                "wfull": wfull,
            }
        )
    return in_maps


class _Runner:
    """Holds the AOT fast-dispatch executable (C++ pjit fast path, no
    donated output: the kernel writes every out element, so PJRT's
    uninitialized result allocation is safe)."""

    def __init__(self):
        import jax
        from jax.experimental.shard_map import shard_map
        from jax.sharding import Mesh, NamedSharding, PartitionSpec

        from concourse import bass2jax
        from concourse.bass2jax import _bass_exec_p, install_neuronx_cc_hook

        try:
            from concourse.bass2jax import fast_dispatch_compile
        except ImportError:
            fast_dispatch_compile = None

        install_neuronx_cc_hook()
        nc = _build_nc()
        self.nc = nc

        out_aval = jax.core.ShapedArray((NSLAB, 128, 8, 8, C), np.dtype(bf16))
        pname = nc.partition_id_tensor.name if nc.partition_id_tensor else None
        all_in = list(IN_NAMES) + ([pname] if pname else [])

        def _body(*args):
            operands = list(args)
            if pname:
                operands.append(bass2jax.partition_id_tensor())
            outs = _bass_exec_p.bind(
                *operands,
                out_avals=(out_aval,),
                in_names=tuple(all_in),
                out_names=("out",),
                lowering_input_output_aliases=(),
                sim_require_finite=True,
                sim_require_nnan=True,
                nc=nc,
            )
            return tuple(outs)

        devices = jax.devices()[:NCORES]
        self.mesh = Mesh(np.asarray(devices), ("core",))
        self.sharding = NamedSharding(self.mesh, PartitionSpec("core"))
        in_shapes = {
            "p1T": (ALL, S), "p1Tq": (ALL, S_CORE), "wqk": (ALL, 2 * ALL),
            "b2d": (128, 12), "relPad2": (128, 2048), "wfull": (128, 512),
        }
        in_dtypes = {n: np.dtype(bf16) for n in IN_NAMES}
        in_dtypes["b2d"] = np.dtype(np.float32)
        arg_structs = [
            jax.ShapeDtypeStruct(
                (NCORES * in_shapes[n][0],) + in_shapes[n][1:],
                in_dtypes[n], sharding=self.sharding)
            for n in IN_NAMES
        ]
        def _compile():
            return jax.jit(
                shard_map(
                    _body, mesh=self.mesh,
                    in_specs=(PartitionSpec("core"),) * len(IN_NAMES),
                    out_specs=(PartitionSpec("core"),), check_rep=False),
                keep_unused=True,
            ).lower(*arg_structs).compile()

        if fast_dispatch_compile is not None:
            self.fast = fast_dispatch_compile(_compile)
        else:
            self.fast = _compile()
        self._jax = jax

    def device_inputs(self, in_maps):
        jax = self._jax
        concat_in = [
            np.concatenate([np.asarray(m[name]) for m in in_maps], axis=0)
            for name in IN_NAMES
        ]
        return [jax.device_put(a, self.sharding) for a in concat_in]

    def run(self, in_maps, out_f32):
        """Execute and write fp32 results directly into out_f32 [B,S,S,C]."""
        dev_in = self.device_inputs(in_maps)
        out_arr = self.fast(*dev_in)[0]

        shards = sorted(out_arr.addressable_shards, key=lambda sh: sh.index[0].start)

        def convert(core, raw):
            b = core // 4
            s_off = (core % 4) * S_CORE
            # fast bf16 -> f32 upcast, then unblock (g,t,u,s,c) -> (s_glob, t_glob, c)
            u = raw.view(np.uint16).astype(np.uint32) << 16
            f = u.view(np.float32)
            out_f32[b, s_off : s_off + S_CORE] = (
                f.transpose(0, 3, 2, 1, 4).reshape(S_CORE, S, C)
            )

        # downloads are serial (tunnel-bound); hide the upcast/detranspose
        # behind the next shard's download via a single worker thread
        from concurrent.futures import ThreadPoolExecutor

        with ThreadPoolExecutor(1) as ex:
            futs = []
            for core, sh in enumerate(shards):
                raw = np.asarray(sh.data)  # network-bound fetch
                futs.append(ex.submit(convert, core, raw))
            for fu in futs:
                fu.result()


def kernel(p0, p1, p2, W_qk, b_qk, rel_emb, W_out, b_out):
    global _COMPILED

    p1 = np.asarray(p1, np.float32)
    W_qk = np.asarray(W_qk, np.float32)
    b_qk = np.asarray(b_qk, np.float32)
    rel_emb = np.asarray(rel_emb, np.float32)
    W_out = np.asarray(W_out, np.float32)
    b_out = np.asarray(b_out, np.float32)

    if _COMPILED is None:
        _COMPILED = _Runner()
    run = _COMPILED

    in_maps = _host_prep(p1, W_qk, b_qk, rel_emb, W_out, b_out)
    full = np.empty((B, S, S, C), np.float32)
    run.run(in_maps, full)
    if np.any(b_out):
        full += np.asarray(b_out, np.float32)
    return full
